# revision 12
# baseline (speedup 1.0000x reference)
"""Trainium2 Bass kernel for nn_PlaneProjection (8-core SPMD).

Math (reference):
    nn = l2norm(normal, axis=1); wn = l2norm(weight, axis=1); xn = l2norm(x, axis=1)
    out = x @ wn.T - (xn @ nn.T) * nw + rw,   nw = sum(nn*wn,1), rw = sum(normal*wn,1)

Rewritten per output-transposed tile (out.T[o, b]):
    out.T = (wn @ x.T) + M ⊙ (nn @ x.T) + rw[:, None],   M[o, b] = -nw[o] / ||x_b||

Sharding: out_features split 8 ways (tensor parallel). Each core computes a
[128, 512] slice of out.T with two K=1024 GEMMs sharing the same moving
operand x.T. Host prepares transposed/normalized operands and the tiny stat
vectors; all O(B*IN*OUT) math runs on device.
"""

import numpy as np

B, IN, OUT = 512, 1024, 1024
P = 128
NCORES = 8
KC = IN // P  # 8 k-chunks
EPS = 1e-12

# GEMM operand dtype: "f32" (exact, 4 cyc/row), "f32r" (relaxed, 1 cyc/row)
MM_DTYPE = "f32r"

_PROGRAM_CACHE = {}


def _patch_tile_drain():
    """Walrus in this container rejects >1 sync-wait on the Tile final Drain
    (CTRL struct). Spread the global-clock waits across SP nops instead."""
    import concourse.tile as tile
    import concourse.mybir as mybir
    from concourse.vector_clock import ScopedClock

    if getattr(tile.TileContext, "_drain_patched", False):
        return

    def _drain_and_barrier(self, tick_clock, wait_clock):
        nc = self.nc
        probe = nc.sync.nop()
        wait_clock.add_sem_waits(probe.ins, ScopedClock({None: tick_clock.global_clock}))
        waits = list(probe.ins.sync_info.on_wait)
        if len(waits) > 1:
            probe.ins.sync_info = mybir.SyncInfo(on_wait=waits[:1], on_update=[])
            for i in range(1, len(waits)):
                n2 = nc.sync.nop()
                n2.ins.sync_info = mybir.SyncInfo(on_wait=waits[i : i + 1], on_update=[])
        nc.sync.drain()
        nc.all_engine_barrier()
        assert self.sems is not None
        popped = nc._tile_sem_poison_stack.pop()
        assert popped is self._sem_poison
        nc.clear_and_free_semaphores(list(self.sems.allocated().values()))
        nc.all_engine_barrier()

    tile.TileContext._drain_and_barrier = _drain_and_barrier
    tile.TileContext._drain_patched = True


def _patch_compile_waitsplit():
    """This container's walrus accepts at most ONE sync-wait per instruction
    (older ISA struct). Rewrite the BIR JSON before compile: excess waits move
    onto same-engine NoOps inserted immediately before the instruction —
    engine program order makes this equivalent."""
    import json

    import concourse.bass_utils as bu
    import concourse.bass2jax as b2j

    if getattr(bu, "_waitsplit_patched", False):
        return
    orig = bu.compile_bir_kernel

    def wrapped(ant_bir_str, *a, **kw):
        d = json.loads(ant_bir_str)
        changed = False
        ctr = 0
        for f in d.get("functions", []):
            for bb in f.get("blocks", []):
                new = []
                for inst in bb.get("instructions", []):
                    si = inst.get("sync_info")
                    waits = (si or {}).get("on_wait") or []
                    if len(waits) > 1:
                        changed = True
                        for w in waits[:-1]:
                            ctr += 1
                            nop = {
                                "engine": inst.get("engine"),
                                "ins": [],
                                "outs": [],
                                "name": f"WSPLIT-{ctr}",
                                "opcode": "NoOp",
                                "sync_info": {"on_update": [], "on_wait": [w]},
                            }
                            if "debug" in inst:
                                nop["debug"] = inst["debug"]
                            new.append(nop)
                        si["on_wait"] = [waits[-1]]
                    new.append(inst)
                bb["instructions"] = new
        if changed:
            ant_bir_str = json.dumps(d).encode()
        return orig(ant_bir_str, *a, **kw)

    bu.compile_bir_kernel = wrapped
    b2j.compile_bir_kernel = wrapped
    bu._waitsplit_patched = True


def _build_program(mm_dtype: str):
    """One SPMD Bass program; per-core data differs, program is identical."""
    if mm_dtype in _PROGRAM_CACHE:
        return _PROGRAM_CACHE[mm_dtype]

    from contextlib import ExitStack

    import concourse.bass as bass
    import concourse.mybir as mybir
    import concourse.tile as tile

    _patch_tile_drain()
    _patch_compile_waitsplit()

    f32 = mybir.dt.float32
    mmdt = f32 if mm_dtype == "f32" else mybir.dt.float32r

    nc = bass.Bass("TRN2", target_bir_lowering=False, debug=False, num_devices=NCORES)

    xT = nc.declare_dram_parameter("xT", [IN, B], mmdt, isOutput=False)
    wnT = nc.declare_dram_parameter("wnT", [IN, P], mmdt, isOutput=False)
    nnT = nc.declare_dram_parameter("nnT", [IN, P], mmdt, isOutput=False)
    m_in = nc.declare_dram_parameter("m", [P, B], f32, isOutput=False)
    rw_in = nc.declare_dram_parameter("rw", [1, P], f32, isOutput=False)
    ones_in = nc.declare_dram_parameter("ones", [1, B], mmdt, isOutput=False)
    out = nc.declare_dram_parameter("out", [P, B], f32, isOutput=True)

    with tile.TileContext(nc) as tc:
        with ExitStack() as ctx:
            sb = ctx.enter_context(tc.tile_pool(name="sb", bufs=1))
            ps = ctx.enter_context(tc.tile_pool(name="ps", bufs=1, space="PSUM"))

            # Stationary operands first (small, unlock matmuls early).
            sw = sb.tile([P, KC, P], mmdt, tag="sw")
            nc.sync.dma_start(sw[:], wnT.rearrange("(k p) o -> p k o", p=P))
            sn = sb.tile([P, KC, P], mmdt, tag="sn")
            nc.sync.dma_start(sn[:], nnT.rearrange("(k p) o -> p k o", p=P))
            srw = sb.tile([1, P], mmdt, tag="srw")
            nc.sync.dma_start(srw[:], rw_in[:].bitcast(mmdt))
            sm = sb.tile([P, B], f32, tag="sm")
            nc.sync.dma_start(sm[:], m_in[:])

            sones = sb.tile([1, B], mmdt, tag="sones")
            nc.sync.dma_start(sones[:], ones_in[:])

            # Moving operand: x.T in 4 chunk-pair tiles (pipeline DMA vs PE).
            sx = []
            xTr = xT.rearrange("(k p) b -> p k b", p=P)  # [128, 8, 512]
            for g in range(4):
                t = sb.tile([P, 2, B], mmdt, tag=f"sx{g}")
                nc.sync.dma_start(t[:], xTr[:, 2 * g : 2 * g + 2, :])
                sx.append(t)

            pa = ps.tile([P, B], f32, tag="pa")
            pb = ps.tile([P, B], f32, tag="pb")

            # A += rw (K=1 ones-row matmul) first: inputs are ready earliest.
            nc.tensor.matmul(pa[:], srw[:], sones[:], start=True, stop=False)
            for k in range(KC):
                xk = sx[k // 2][:, k % 2, :]
                wk = sw[:, k, :]
                nk = sn[:, k, :]
                nc.tensor.matmul(pa[:], wk, xk, start=False, stop=(k == KC - 1))
                nc.tensor.matmul(pb[:], nk, xk, start=(k == 0), stop=(k == KC - 1))

            st = sb.tile([P, B], f32, tag="st")
            nc.vector.tensor_mul(st[:], pb[:], sm[:])
            so = sb.tile([P, B], f32, tag="so")
            nc.vector.tensor_add(so[:], pa[:], st[:])
            nc.sync.dma_start(out[:], so[:])

    _PROGRAM_CACHE[mm_dtype] = nc
    return nc


def _host_prep(x, normal, weight):
    """Normalize/transpose operands and build per-core input maps."""
    x = np.ascontiguousarray(np.asarray(x, dtype=np.float32))
    normal = np.asarray(normal, dtype=np.float32)
    weight = np.asarray(weight, dtype=np.float32)

    w64 = weight.astype(np.float64)
    n64 = normal.astype(np.float64)
    x64 = x.astype(np.float64)

    wnorm = np.maximum(np.sqrt(np.sum(w64 * w64, axis=1)), EPS)  # [OUT]
    nnorm = np.maximum(np.sqrt(np.sum(n64 * n64, axis=1)), EPS)  # [OUT]
    xnorm = np.maximum(np.sqrt(np.sum(x64 * x64, axis=1)), EPS)  # [B]

    wn64 = w64 / wnorm[:, None]
    nn64 = n64 / nnorm[:, None]
    nw = np.sum(nn64 * wn64, axis=1)  # [OUT]
    rw = np.sum(n64 * wn64, axis=1)  # [OUT]

    wnT = np.ascontiguousarray(wn64.T.astype(np.float32))  # [IN, OUT]
    nnT = np.ascontiguousarray(nn64.T.astype(np.float32))  # [IN, OUT]
    xT = np.ascontiguousarray(x.T)  # [IN, B]
    s = (1.0 / xnorm).astype(np.float64)  # [B]

    in_maps = []
    for c in range(NCORES):
        o0 = c * P
        m_c = (-nw[o0 : o0 + P, None] * s[None, :]).astype(np.float32)  # [P, B]
        in_maps.append(
            {
                "xT": xT,
                "wnT": np.ascontiguousarray(wnT[:, o0 : o0 + P]),
                "nnT": np.ascontiguousarray(nnT[:, o0 : o0 + P]),
                "m": m_c,
                "rw": rw[o0 : o0 + P].astype(np.float32).reshape(1, P),
                "ones": np.ones((1, B), dtype=np.float32),
            }
        )
    return in_maps


def kernel(x, normal, weight):
    from concourse.bass_utils import run_bass_kernel_spmd

    in_maps = _host_prep(x, normal, weight)
    nc = _build_program(MM_DTYPE)
    res = run_bass_kernel_spmd(nc, in_maps, core_ids=list(range(NCORES)))
    outT = np.concatenate([res.results[c]["out"] for c in range(NCORES)], axis=0)
    return np.ascontiguousarray(outT.T)


# revision 13
# speedup vs baseline: 1.0478x; 1.0478x over previous
"""Trainium2 Bass kernel for nn_PlaneProjection (8-core SPMD).

Math (reference):
    nn = l2norm(normal, axis=1); wn = l2norm(weight, axis=1); xn = l2norm(x, axis=1)
    out = x @ wn.T - (xn @ nn.T) * nw + rw,   nw = sum(nn*wn,1), rw = sum(normal*wn,1)

Rewritten per output-transposed tile (out.T[o, b]):
    out.T = (wn @ x.T + rw[:, None]) + M ⊙ (nn @ x.T),   M[o, b] = -nw[o] / ||x_b||

Sharding: out_features split 8 ways (tensor parallel). Each core computes a
[128, 512] slice of out.T with two K=1024 GEMMs sharing the same moving
operand x.T. The rw bias is folded into the A-GEMM as a K=1 ones-row matmul;
M is built on-chip as a K=1 outer-product matmul. Host prepares transposed /
normalized operands and the tiny stat vectors; all O(B*IN*OUT) math runs on
device in float32r (fp32 storage, relaxed-precision multiply, fp32 accum).
"""

import numpy as np

B, IN, OUT = 512, 1024, 1024
P = 128
NCORES = 8
KC = IN // P  # 8 k-chunks
EPS = 1e-12

# GEMM operand dtype: "f32" (exact, 4 cyc/row), "f32r" (relaxed, 1 cyc/row)
MM_DTYPE = "f32r"

_PROGRAM_CACHE = {}


def _patch_tile_drain():
    """Walrus in this container rejects >1 sync-wait on the Tile final Drain
    (CTRL struct). Spread the global-clock waits across SP nops instead."""
    import concourse.tile as tile
    import concourse.mybir as mybir
    from concourse.vector_clock import ScopedClock

    if getattr(tile.TileContext, "_drain_patched", False):
        return

    def _drain_and_barrier(self, tick_clock, wait_clock):
        nc = self.nc
        probe = nc.sync.nop()
        wait_clock.add_sem_waits(probe.ins, ScopedClock({None: tick_clock.global_clock}))
        waits = list(probe.ins.sync_info.on_wait)
        if len(waits) > 1:
            probe.ins.sync_info = mybir.SyncInfo(on_wait=waits[:1], on_update=[])
            for i in range(1, len(waits)):
                n2 = nc.sync.nop()
                n2.ins.sync_info = mybir.SyncInfo(on_wait=waits[i : i + 1], on_update=[])
        nc.sync.drain()
        nc.all_engine_barrier()
        assert self.sems is not None
        popped = nc._tile_sem_poison_stack.pop()
        assert popped is self._sem_poison
        nc.clear_and_free_semaphores(list(self.sems.allocated().values()))
        nc.all_engine_barrier()

    tile.TileContext._drain_and_barrier = _drain_and_barrier
    tile.TileContext._drain_patched = True


def _patch_compile_waitsplit():
    """This container's walrus accepts at most ONE sync-wait per instruction
    (older ISA struct). Rewrite the BIR JSON before compile: excess waits move
    onto same-engine NoOps inserted immediately before the instruction —
    engine program order makes this equivalent."""
    import json

    import concourse.bass_utils as bu
    import concourse.bass2jax as b2j

    if getattr(bu, "_waitsplit_patched", False):
        return
    orig = bu.compile_bir_kernel

    def wrapped(ant_bir_str, *a, **kw):
        d = json.loads(ant_bir_str)
        changed = False
        ctr = 0
        for f in d.get("functions", []):
            for bb in f.get("blocks", []):
                new = []
                for inst in bb.get("instructions", []):
                    si = inst.get("sync_info")
                    waits = (si or {}).get("on_wait") or []
                    if len(waits) > 1:
                        changed = True
                        for w in waits[:-1]:
                            ctr += 1
                            nop = {
                                "engine": inst.get("engine"),
                                "ins": [],
                                "outs": [],
                                "name": f"WSPLIT-{ctr}",
                                "opcode": "NoOp",
                                "sync_info": {"on_update": [], "on_wait": [w]},
                            }
                            if "debug" in inst:
                                nop["debug"] = inst["debug"]
                            new.append(nop)
                        si["on_wait"] = [waits[-1]]
                    new.append(inst)
                bb["instructions"] = new
        if changed:
            ant_bir_str = json.dumps(d).encode()
        return orig(ant_bir_str, *a, **kw)

    bu.compile_bir_kernel = wrapped
    b2j.compile_bir_kernel = wrapped
    bu._waitsplit_patched = True


def _build_program(mm_dtype: str):
    """One SPMD Bass program; per-core data differs, program is identical."""
    if mm_dtype in _PROGRAM_CACHE:
        return _PROGRAM_CACHE[mm_dtype]

    from contextlib import ExitStack

    import concourse.bass as bass
    import concourse.mybir as mybir
    import concourse.tile as tile

    _patch_tile_drain()
    _patch_compile_waitsplit()

    f32 = mybir.dt.float32
    mmdt = f32 if mm_dtype == "f32" else mybir.dt.float32r

    nc = bass.Bass("TRN2", target_bir_lowering=False, debug=False, num_devices=NCORES)

    xT = nc.declare_dram_parameter("xT", [IN, B], mmdt, isOutput=False)
    wnT = nc.declare_dram_parameter("wnT", [IN, P], mmdt, isOutput=False)
    nnT = nc.declare_dram_parameter("nnT", [IN, P], mmdt, isOutput=False)
    rw_in = nc.declare_dram_parameter("rw", [1, P], mmdt, isOutput=False)
    nwneg_in = nc.declare_dram_parameter("nwneg", [1, P], mmdt, isOutput=False)
    srow_in = nc.declare_dram_parameter("srow", [1, B], mmdt, isOutput=False)
    ones_in = nc.declare_dram_parameter("ones", [1, B], mmdt, isOutput=False)
    out = nc.declare_dram_parameter("out", [P, B], f32, isOutput=True)

    with tile.TileContext(nc) as tc:
        with ExitStack() as ctx:
            sb = ctx.enter_context(tc.tile_pool(name="sb", bufs=1))
            ps = ctx.enter_context(tc.tile_pool(name="ps", bufs=1, space="PSUM"))

            # Tiny vectors first (unlock the K=1 matmuls immediately).
            srw = sb.tile([1, P], mmdt, tag="srw")
            nc.sync.dma_start(srw[:], rw_in[:])
            snw = sb.tile([1, P], mmdt, tag="snw")
            nc.sync.dma_start(snw[:], nwneg_in[:])
            ssr = sb.tile([1, B], mmdt, tag="ssr")
            nc.sync.dma_start(ssr[:], srow_in[:])
            sones = sb.tile([1, B], mmdt, tag="sones")
            nc.sync.dma_start(sones[:], ones_in[:])

            # Stationary operands (sw on sync ring, sn on scalar ring).
            sw = sb.tile([P, KC, P], mmdt, tag="sw")
            nc.sync.dma_start(sw[:], wnT.rearrange("(k p) o -> p k o", p=P))
            sn = sb.tile([P, KC, P], mmdt, tag="sn")
            nc.scalar.dma_start(sn[:], nnT.rearrange("(k p) o -> p k o", p=P))

            # Moving operand: x.T streamed chunk-by-chunk, alternating rings.
            xTr = xT.rearrange("(k p) b -> p k b", p=P)  # [128, 8, 512]
            sx = []
            for k in range(KC):
                t = sb.tile([P, B], mmdt, tag=f"sx{k}")
                eng = nc.sync if k % 2 == 0 else nc.scalar
                eng.dma_start(t[:], xTr[:, k, :])
                sx.append(t)

            pm = ps.tile([P, B], f32, tag="pm")
            pa = ps.tile([P, B], f32, tag="pa")
            pb = ps.tile([P, B], f32, tag="pb")

            # M = outer(-nw, s) on PE, copied to SBUF by the scalar engine.
            nc.tensor.matmul(pm[:], snw[:], ssr[:], start=True, stop=True)
            sm = sb.tile([P, B], f32, tag="sm")
            nc.scalar.copy(sm[:], pm[:])

            # A += rw broadcast (K=1 ones-row); inputs ready earliest.
            nc.tensor.matmul(pa[:], srw[:], sones[:], start=True, stop=False)
            for k in range(KC):
                nc.tensor.matmul(pb[:], sn[:, k, :], sx[k][:], start=(k == 0),
                                 stop=(k == KC - 1))
                nc.tensor.matmul(pa[:], sw[:, k, :], sx[k][:], start=False,
                                 stop=(k == KC - 1))

            st = sb.tile([P, B], f32, tag="st")
            nc.vector.tensor_mul(st[:], pb[:], sm[:])
            so = sb.tile([P, B], f32, tag="so")
            nc.vector.tensor_add(so[:], pa[:], st[:])
            nc.sync.dma_start(out[:], so[:])

    _PROGRAM_CACHE[mm_dtype] = nc
    return nc


def _host_prep(x, normal, weight):
    """Normalize/transpose operands and build per-core input maps."""
    x = np.ascontiguousarray(np.asarray(x, dtype=np.float32))
    normal = np.asarray(normal, dtype=np.float32)
    weight = np.asarray(weight, dtype=np.float32)

    w64 = weight.astype(np.float64)
    n64 = normal.astype(np.float64)
    x64 = x.astype(np.float64)

    wnorm = np.maximum(np.sqrt(np.sum(w64 * w64, axis=1)), EPS)  # [OUT]
    nnorm = np.maximum(np.sqrt(np.sum(n64 * n64, axis=1)), EPS)  # [OUT]
    xnorm = np.maximum(np.sqrt(np.sum(x64 * x64, axis=1)), EPS)  # [B]

    wn64 = w64 / wnorm[:, None]
    nn64 = n64 / nnorm[:, None]
    nw = np.sum(nn64 * wn64, axis=1)  # [OUT]
    rw = np.sum(n64 * wn64, axis=1)  # [OUT]

    wnT = np.ascontiguousarray(wn64.T.astype(np.float32))  # [IN, OUT]
    nnT = np.ascontiguousarray(nn64.T.astype(np.float32))  # [IN, OUT]
    xT = np.ascontiguousarray(x.T)  # [IN, B]
    srow = (1.0 / xnorm).astype(np.float32).reshape(1, B)
    ones = np.ones((1, B), dtype=np.float32)

    in_maps = []
    for c in range(NCORES):
        o0 = c * P
        in_maps.append(
            {
                "xT": xT,
                "wnT": np.ascontiguousarray(wnT[:, o0 : o0 + P]),
                "nnT": np.ascontiguousarray(nnT[:, o0 : o0 + P]),
                "rw": rw[o0 : o0 + P].astype(np.float32).reshape(1, P),
                "nwneg": (-nw[o0 : o0 + P]).astype(np.float32).reshape(1, P),
                "srow": srow,
                "ones": ones,
            }
        )
    return in_maps


def kernel(x, normal, weight):
    from concourse.bass_utils import run_bass_kernel_spmd

    in_maps = _host_prep(x, normal, weight)
    nc = _build_program(MM_DTYPE)
    res = run_bass_kernel_spmd(nc, in_maps, core_ids=list(range(NCORES)))
    outT = np.concatenate([res.results[c]["out"] for c in range(NCORES)], axis=0)
    return np.ascontiguousarray(outT.T)


# revision 15
# speedup vs baseline: 1.0868x; 1.0372x over previous
"""Trainium2 Bass kernel for nn_PlaneProjection (8-core SPMD).

Math (reference):
    nn = l2norm(normal, axis=1); wn = l2norm(weight, axis=1); xn = l2norm(x, axis=1)
    out = x @ wn.T - (xn @ nn.T) * nw + rw,   nw = sum(nn*wn,1), rw = sum(normal*wn,1)

Rewritten per output-transposed tile (out.T[o, b]):
    out.T = (wn @ x.T + rw[:, None]) + M ⊙ (nn @ x.T),   M[o, b] = -nw[o] / ||x_b||

Sharding: out_features split 8 ways (tensor parallel). Each core computes a
[128, 512] slice of out.T with two K=1024 GEMMs sharing the same moving
operand x.T. The rw bias is folded into the A-GEMM as a K=1 ones-row matmul;
M is built on-chip as a K=1 outer-product matmul. Host prepares transposed /
normalized operands and the tiny stat vectors; all O(B*IN*OUT) math runs on
device in float32r (fp32 storage, relaxed-precision multiply, fp32 accum).
"""

import numpy as np

B, IN, OUT = 512, 1024, 1024
P = 128
NCORES = 8
KC = IN // P  # 8 k-chunks
EPS = 1e-12

# GEMM operand dtype: "f32" (exact, 4 cyc/row), "f32r" (relaxed, 1 cyc/row)
MM_DTYPE = "f32r"

_PROGRAM_CACHE = {}


def _patch_tile_drain():
    """Walrus in this container rejects >1 sync-wait on the Tile final Drain
    (CTRL struct). Spread the global-clock waits across SP nops instead."""
    import concourse.tile as tile
    import concourse.mybir as mybir
    from concourse.vector_clock import ScopedClock

    if getattr(tile.TileContext, "_drain_patched", False):
        return

    def _drain_and_barrier(self, tick_clock, wait_clock):
        nc = self.nc
        probe = nc.sync.nop()
        wait_clock.add_sem_waits(probe.ins, ScopedClock({None: tick_clock.global_clock}))
        waits = list(probe.ins.sync_info.on_wait)
        if len(waits) > 1:
            probe.ins.sync_info = mybir.SyncInfo(on_wait=waits[:1], on_update=[])
            for i in range(1, len(waits)):
                n2 = nc.sync.nop()
                n2.ins.sync_info = mybir.SyncInfo(on_wait=waits[i : i + 1], on_update=[])
        nc.sync.drain()
        nc.all_engine_barrier()
        assert self.sems is not None
        popped = nc._tile_sem_poison_stack.pop()
        assert popped is self._sem_poison
        nc.clear_and_free_semaphores(list(self.sems.allocated().values()))
        nc.all_engine_barrier()

    tile.TileContext._drain_and_barrier = _drain_and_barrier
    tile.TileContext._drain_patched = True


def _patch_compile_waitsplit():
    """This container's walrus accepts at most ONE sync-wait per instruction
    (older ISA struct). Rewrite the BIR JSON before compile: excess waits move
    onto same-engine NoOps inserted immediately before the instruction —
    engine program order makes this equivalent."""
    import json

    import concourse.bass_utils as bu
    import concourse.bass2jax as b2j

    if getattr(bu, "_waitsplit_patched", False):
        return
    orig = bu.compile_bir_kernel

    def wrapped(ant_bir_str, *a, **kw):
        d = json.loads(ant_bir_str)
        changed = False
        ctr = 0
        for f in d.get("functions", []):
            for bb in f.get("blocks", []):
                new = []
                for inst in bb.get("instructions", []):
                    si = inst.get("sync_info")
                    waits = (si or {}).get("on_wait") or []
                    if len(waits) > 1:
                        changed = True
                        for w in waits[:-1]:
                            ctr += 1
                            nop = {
                                "engine": inst.get("engine"),
                                "ins": [],
                                "outs": [],
                                "name": f"WSPLIT-{ctr}",
                                "opcode": "NoOp",
                                "sync_info": {"on_update": [], "on_wait": [w]},
                            }
                            if "debug" in inst:
                                nop["debug"] = inst["debug"]
                            new.append(nop)
                        si["on_wait"] = [waits[-1]]
                    new.append(inst)
                bb["instructions"] = new
        if changed:
            ant_bir_str = json.dumps(d).encode()
        return orig(ant_bir_str, *a, **kw)

    bu.compile_bir_kernel = wrapped
    b2j.compile_bir_kernel = wrapped
    bu._waitsplit_patched = True


def _build_program(mm_dtype: str):
    """One SPMD Bass program; per-core data differs, program is identical."""
    if mm_dtype in _PROGRAM_CACHE:
        return _PROGRAM_CACHE[mm_dtype]

    from contextlib import ExitStack

    import concourse.bass as bass
    import concourse.mybir as mybir
    import concourse.tile as tile

    _patch_tile_drain()
    _patch_compile_waitsplit()

    f32 = mybir.dt.float32
    mmdt = f32 if mm_dtype == "f32" else mybir.dt.float32r

    nc = bass.Bass("TRN2", target_bir_lowering=False, debug=False, num_devices=NCORES)

    VEC = 2 * P + 2 * B  # [rw | nwneg | srow | ones]
    xT = nc.declare_dram_parameter("xT", [IN, B], mmdt, isOutput=False)
    wnT = nc.declare_dram_parameter("wnT", [IN, P], mmdt, isOutput=False)
    nnT = nc.declare_dram_parameter("nnT", [IN, P], mmdt, isOutput=False)
    vecs_in = nc.declare_dram_parameter("vecs", [1, VEC], mmdt, isOutput=False)
    out = nc.declare_dram_parameter("out", [P, B], f32, isOutput=True)

    with tile.TileContext(nc) as tc:
        with ExitStack() as ctx:
            sb = ctx.enter_context(tc.tile_pool(name="sb", bufs=1))
            ps = ctx.enter_context(tc.tile_pool(name="ps", bufs=1, space="PSUM"))

            # Packed tiny vectors first (unlock the K=1 matmuls immediately).
            sv = sb.tile([1, VEC], mmdt, tag="sv")
            nc.sync.dma_start(sv[:], vecs_in[:])
            srw = sv[:, 0:P]
            snw = sv[:, P : 2 * P]
            ssr = sv[:, 2 * P : 2 * P + B]
            sones = sv[:, 2 * P + B : VEC]

            # Stationary operands (sw on sync ring, sn on scalar ring).
            sw = sb.tile([P, KC, P], mmdt, tag="sw")
            nc.sync.dma_start(sw[:], wnT.rearrange("(k p) o -> p k o", p=P))
            sn = sb.tile([P, KC, P], mmdt, tag="sn")
            nc.scalar.dma_start(sn[:], nnT.rearrange("(k p) o -> p k o", p=P))

            # Moving operand: x.T streamed in 4 chunk-pair tiles, alternating
            # HWDGE rings so the two halves load concurrently.
            xTr = xT.rearrange("(k p) b -> p k b", p=P)  # [128, 8, 512]
            sx = []
            for g in range(KC // 2):
                t = sb.tile([P, 2, B], mmdt, tag=f"sx{g}")
                eng = nc.scalar if g % 2 == 0 else nc.sync
                eng.dma_start(t[:], xTr[:, 2 * g : 2 * g + 2, :])
                sx.append(t)

            pm = ps.tile([P, B], f32, tag="pm")
            pa = ps.tile([P, B], f32, tag="pa")
            pb = ps.tile([P, B], f32, tag="pb")

            # M = outer(-nw, s) on PE, copied to SBUF by the scalar engine.
            nc.tensor.matmul(pm[:], snw, ssr, start=True, stop=True)
            sm = sb.tile([P, B], f32, tag="sm")
            nc.scalar.copy(sm[:], pm[:])

            # A += rw broadcast (K=1 ones-row); inputs ready earliest.
            nc.tensor.matmul(pa[:], srw, sones, start=True, stop=False)
            for k in range(KC):
                xk = sx[k // 2][:, k % 2, :]
                nc.tensor.matmul(pb[:], sn[:, k, :], xk, start=(k == 0),
                                 stop=(k == KC - 1))
                nc.tensor.matmul(pa[:], sw[:, k, :], xk, start=False,
                                 stop=(k == KC - 1))

            # Epilogue + store split in column halves so the first out-DMA
            # overlaps the second half's vector work.
            H = B // 2
            st = sb.tile([P, B], f32, tag="st")
            so = sb.tile([P, B], f32, tag="so")
            for h in range(2):
                c = slice(h * H, (h + 1) * H)
                nc.vector.tensor_mul(st[:, c], pb[:, c], sm[:, c])
                nc.vector.tensor_add(so[:, c], pa[:, c], st[:, c])
                eng = nc.sync if h == 0 else nc.scalar
                eng.dma_start(out[:, c], so[:, c])

    _PROGRAM_CACHE[mm_dtype] = nc
    return nc


def _host_prep(x, normal, weight):
    """Normalize/transpose operands and build per-core input maps."""
    x = np.ascontiguousarray(np.asarray(x, dtype=np.float32))
    normal = np.asarray(normal, dtype=np.float32)
    weight = np.asarray(weight, dtype=np.float32)

    w64 = weight.astype(np.float64)
    n64 = normal.astype(np.float64)
    x64 = x.astype(np.float64)

    wnorm = np.maximum(np.sqrt(np.sum(w64 * w64, axis=1)), EPS)  # [OUT]
    nnorm = np.maximum(np.sqrt(np.sum(n64 * n64, axis=1)), EPS)  # [OUT]
    xnorm = np.maximum(np.sqrt(np.sum(x64 * x64, axis=1)), EPS)  # [B]

    wn64 = w64 / wnorm[:, None]
    nn64 = n64 / nnorm[:, None]
    nw = np.sum(nn64 * wn64, axis=1)  # [OUT]
    rw = np.sum(n64 * wn64, axis=1)  # [OUT]

    wnT = np.ascontiguousarray(wn64.T.astype(np.float32))  # [IN, OUT]
    nnT = np.ascontiguousarray(nn64.T.astype(np.float32))  # [IN, OUT]
    xT = np.ascontiguousarray(x.T)  # [IN, B]
    srow = (1.0 / xnorm).astype(np.float32)
    ones = np.ones(B, dtype=np.float32)

    in_maps = []
    for c in range(NCORES):
        o0 = c * P
        vecs = np.concatenate(
            [
                rw[o0 : o0 + P].astype(np.float32),
                (-nw[o0 : o0 + P]).astype(np.float32),
                srow,
                ones,
            ]
        ).reshape(1, -1)
        in_maps.append(
            {
                "xT": xT,
                "wnT": np.ascontiguousarray(wnT[:, o0 : o0 + P]),
                "nnT": np.ascontiguousarray(nnT[:, o0 : o0 + P]),
                "vecs": vecs,
            }
        )
    return in_maps


def kernel(x, normal, weight):
    from concourse.bass_utils import run_bass_kernel_spmd

    in_maps = _host_prep(x, normal, weight)
    nc = _build_program(MM_DTYPE)
    res = run_bass_kernel_spmd(nc, in_maps, core_ids=list(range(NCORES)))
    outT = np.concatenate([res.results[c]["out"] for c in range(NCORES)], axis=0)
    return np.ascontiguousarray(outT.T)


# revision 18
# speedup vs baseline: 31320.9986x; 28818.7712x over previous
"""Trainium2 Bass kernel for nn_PlaneProjection (8-core SPMD).

Math (reference):
    nn = l2norm(normal, axis=1); wn = l2norm(weight, axis=1); xn = l2norm(x, axis=1)
    out = x @ wn.T - (xn @ nn.T) * nw + rw,   nw = sum(nn*wn,1), rw = sum(normal*wn,1)

Rewritten per output-transposed tile (out.T[o, b]):
    out.T = (wn @ x.T + rw[:, None]) + M ⊙ (nn @ x.T),   M[o, b] = -nw[o] / ||x_b||

Sharding: out_features split 8 ways (tensor parallel). Each core computes a
[128, 512] slice of out.T with two K=1024 GEMMs sharing the same moving
operand x.T, in float32r (fp32 storage, relaxed multiply, fp32 accumulate).

Device data layout: one packed "blob" [1024, 768] = [x.T | wn_slice.T |
nn_slice.T] streamed as a few large chunk DMAs split across the two HWDGE
rings (DMA count dominates; bytes are the roofline), plus one tiny packed
vector DMA [rw | -nw | 1/||x_b|| | ones]. rw enters the A-GEMM as a K=1
ones-row matmul; M is built on-chip as a K=1 outer-product matmul. Host does
layout prep and O(N^2) stats only; all O(B*IN*OUT) math runs on device.
"""

import numpy as np

B, IN, OUT = 512, 1024, 1024
P = 128
NCORES = 8
KC = IN // P  # 8 k-chunks of the contraction
EPS = 1e-12
W0, N0 = B, B + P  # blob column offsets
CW = B + 2 * P  # blob width: 768
VEC = 2 * P + 2 * B  # packed vec width

# GEMM operand dtype: "f32" (exact, 4 cyc/row), "f32r" (relaxed, 1 cyc/row)
MM_DTYPE = "f32r"
# (k-chunk count, issuing ring) per blob DMA — tuned on hardware.
KSPLITS = ((3, "scalar"), (3, "sync"), (2, "scalar"))
VECS_RING = "scalar"

_PROGRAM_CACHE = {}


def _patch_tile_drain():
    """Walrus in this container rejects >1 sync-wait on the Tile final Drain
    (CTRL struct). Spread the global-clock waits across SP nops instead."""
    import concourse.tile as tile
    import concourse.mybir as mybir
    from concourse.vector_clock import ScopedClock

    if getattr(tile.TileContext, "_drain_patched", False):
        return

    def _drain_and_barrier(self, tick_clock, wait_clock):
        nc = self.nc
        probe = nc.sync.nop()
        wait_clock.add_sem_waits(probe.ins, ScopedClock({None: tick_clock.global_clock}))
        waits = list(probe.ins.sync_info.on_wait) if probe.ins.sync_info else []
        if len(waits) > 1:
            probe.ins.sync_info = mybir.SyncInfo(on_wait=waits[:1], on_update=[])
            for i in range(1, len(waits)):
                n2 = nc.sync.nop()
                n2.ins.sync_info = mybir.SyncInfo(on_wait=waits[i : i + 1], on_update=[])
        nc.sync.drain()
        nc.all_engine_barrier()
        assert self.sems is not None
        popped = nc._tile_sem_poison_stack.pop()
        assert popped is self._sem_poison
        nc.clear_and_free_semaphores(list(self.sems.allocated().values()))
        nc.all_engine_barrier()

    tile.TileContext._drain_and_barrier = _drain_and_barrier
    tile.TileContext._drain_patched = True


def _patch_compile_waitsplit():
    """This container's walrus accepts at most ONE sync-wait per instruction
    (older ISA struct). Rewrite the BIR JSON before compile: excess waits move
    onto same-engine NoOps inserted immediately before the instruction —
    engine program order makes this equivalent. Also disable the in-compiler
    BIR simulator (costs minutes, not needed to produce the NEFF)."""
    import json

    import concourse.bass_utils as bu
    import concourse.bass2jax as b2j

    if getattr(bu, "_waitsplit_patched", False):
        return
    orig = bu.compile_bir_kernel

    def wrapped(ant_bir_str, *a, **kw):
        d = json.loads(ant_bir_str)
        changed = False
        ctr = 0
        for f in d.get("functions", []):
            for bb in f.get("blocks", []):
                new = []
                for inst in bb.get("instructions", []):
                    si = inst.get("sync_info")
                    waits = (si or {}).get("on_wait") or []
                    if len(waits) > 1:
                        changed = True
                        for w in waits[:-1]:
                            ctr += 1
                            nop = {
                                "engine": inst.get("engine"),
                                "ins": [],
                                "outs": [],
                                "name": f"WSPLIT-{ctr}",
                                "opcode": "NoOp",
                                "sync_info": {"on_update": [], "on_wait": [w]},
                            }
                            if "debug" in inst:
                                nop["debug"] = inst["debug"]
                            new.append(nop)
                        si["on_wait"] = [waits[-1]]
                    new.append(inst)
                bb["instructions"] = new
        if changed:
            ant_bir_str = json.dumps(d).encode()
        return orig(ant_bir_str, *a, **kw)

    bu.compile_bir_kernel = wrapped
    b2j.compile_bir_kernel = wrapped
    bu._waitsplit_patched = True

    orig_run = bu.run_command

    def run2(cmd, **kw):
        if isinstance(cmd, list):
            cmd = [
                c.replace("--enable-birsim=true", "--enable-birsim=false")
                if isinstance(c, str)
                else c
                for c in cmd
            ]
        return orig_run(cmd, **kw)

    bu.run_command = run2


def _build_program(mm_dtype: str):
    """One SPMD Bass program; per-core data differs, program is identical."""
    if mm_dtype in _PROGRAM_CACHE:
        return _PROGRAM_CACHE[mm_dtype]

    from contextlib import ExitStack

    import concourse.bass as bass
    import concourse.mybir as mybir
    import concourse.tile as tile

    _patch_tile_drain()
    _patch_compile_waitsplit()

    f32 = mybir.dt.float32
    mmdt = f32 if mm_dtype == "f32" else mybir.dt.float32r

    nc = bass.Bass("TRN2", target_bir_lowering=False, debug=False, num_devices=NCORES)

    blob = nc.declare_dram_parameter("blob", [IN, CW], mmdt, isOutput=False)
    vecs_in = nc.declare_dram_parameter("vecs", [1, VEC], mmdt, isOutput=False)
    out = nc.declare_dram_parameter("out", [P, B], f32, isOutput=True)
    blr = blob.rearrange("(k p) c -> p k c", p=P)  # [128, 8, 768]

    with tile.TileContext(nc) as tc:
        with ExitStack() as ctx:
            sb = ctx.enter_context(tc.tile_pool(name="sb", bufs=1))
            ps = ctx.enter_context(tc.tile_pool(name="ps", bufs=1, space="PSUM"))
            engs = {"sync": nc.sync, "scalar": nc.scalar}

            # Packed tiny vectors (unlock the K=1 matmuls immediately).
            sv = sb.tile([1, VEC], mmdt, tag="sv")
            engs[VECS_RING].dma_start(sv[:], vecs_in[:])
            srw = sv[:, 0:P]
            snw = sv[:, P : 2 * P]
            ssr = sv[:, 2 * P : 2 * P + B]
            sones = sv[:, 2 * P + B : VEC]

            # Blob chunks across the two HWDGE rings.
            chunks = {}
            k0 = 0
            for g, (kn, ring) in enumerate(KSPLITS):
                t = sb.tile([P, kn, CW], mmdt, tag=f"ch{g}")
                engs[ring].dma_start(t[:], blr[:, k0 : k0 + kn, :])
                for i in range(kn):
                    chunks[k0 + i] = (t, i)
                k0 += kn
            assert k0 == KC

            def sl(k, c0, c1):
                t, i = chunks[k]
                return t[:, i, c0:c1]

            pm = ps.tile([P, B], f32, tag="pm")
            pa = ps.tile([P, B], f32, tag="pa")
            pb = ps.tile([P, B], f32, tag="pb")

            # M = outer(-nw, s) on PE, copied to SBUF by the scalar engine.
            nc.tensor.matmul(pm[:], snw, ssr, start=True, stop=True)
            sm = sb.tile([P, B], f32, tag="sm")
            nc.scalar.copy(sm[:], pm[:])

            # A += rw broadcast (K=1 ones-row); inputs ready earliest.
            nc.tensor.matmul(pa[:], srw, sones, start=True, stop=False)
            for k in range(KC):
                nc.tensor.matmul(pb[:], sl(k, N0, N0 + P), sl(k, 0, B),
                                 start=(k == 0), stop=(k == KC - 1))
                nc.tensor.matmul(pa[:], sl(k, W0, W0 + P), sl(k, 0, B),
                                 start=False, stop=(k == KC - 1))

            st = sb.tile([P, B], f32, tag="st")
            nc.vector.tensor_mul(st[:], pb[:], sm[:])
            so = sb.tile([P, B], f32, tag="so")
            nc.vector.tensor_add(so[:], pa[:], st[:])
            nc.sync.dma_start(out[:], so[:])

    _PROGRAM_CACHE[mm_dtype] = nc
    return nc


def _host_prep(x, normal, weight):
    """Normalize/transpose operands and build per-core input maps."""
    x = np.ascontiguousarray(np.asarray(x, dtype=np.float32))
    normal = np.asarray(normal, dtype=np.float32)
    weight = np.asarray(weight, dtype=np.float32)

    w64 = weight.astype(np.float64)
    n64 = normal.astype(np.float64)
    x64 = x.astype(np.float64)

    wnorm = np.maximum(np.sqrt(np.sum(w64 * w64, axis=1)), EPS)  # [OUT]
    nnorm = np.maximum(np.sqrt(np.sum(n64 * n64, axis=1)), EPS)  # [OUT]
    xnorm = np.maximum(np.sqrt(np.sum(x64 * x64, axis=1)), EPS)  # [B]

    wn64 = w64 / wnorm[:, None]
    nn64 = n64 / nnorm[:, None]
    nw = np.sum(nn64 * wn64, axis=1)  # [OUT]
    rw = np.sum(n64 * wn64, axis=1)  # [OUT]

    wnT = wn64.T.astype(np.float32)  # [IN, OUT]
    nnT = nn64.T.astype(np.float32)  # [IN, OUT]
    xT = x.T  # [IN, B]
    srow = (1.0 / xnorm).astype(np.float32)
    ones = np.ones(B, dtype=np.float32)

    in_maps = []
    for c in range(NCORES):
        o0 = c * P
        blob = np.empty((IN, CW), np.float32)
        blob[:, 0:B] = xT
        blob[:, W0 : W0 + P] = wnT[:, o0 : o0 + P]
        blob[:, N0 : N0 + P] = nnT[:, o0 : o0 + P]
        vecs = np.concatenate(
            [
                rw[o0 : o0 + P].astype(np.float32),
                (-nw[o0 : o0 + P]).astype(np.float32),
                srow,
                ones,
            ]
        ).reshape(1, VEC)
        in_maps.append({"blob": blob, "vecs": vecs})
    return in_maps


def kernel(x, normal, weight):
    from concourse.bass_utils import run_bass_kernel_spmd

    in_maps = _host_prep(x, normal, weight)
    nc = _build_program(MM_DTYPE)
    res = run_bass_kernel_spmd(nc, in_maps, core_ids=list(range(NCORES)))
    outT = np.concatenate([res.results[c]["out"] for c in range(NCORES)], axis=0)
    return np.ascontiguousarray(outT.T)
